# revision 11
# baseline (speedup 1.0000x reference)
"""MoE (dense-act-dense, top-4 of 8 experts) Trainium2 kernel.

Strategy (expert-parallel, host-side dispatch):
  - The forward combine weight is exactly 1.0 (straight-through gate trick in
    the reference), so out[n] = sum_{e in top4(n)} expert_e(x[n]).
  - Host computes the tiny gate matmul + top-4 routing (0.05% of FLOPs) and
    dispatches tokens: core e receives the tokens routed to expert e
    (capacity-padded), plus expert e's weights. This is the sharding step.
  - Each of the 8 cores runs a dense 2-layer MLP (relu between) on its tokens:
      h = relu(w1[e] @ x) ; y = w2[e] @ h
    as two chained GEMMs in bf16 (fp32 PSUM accumulate).
  - Host scatter-adds per-expert outputs back (weight 1.0 per selection).

Why bf16 (not fp32r): trace analysis showed the fp32r kernel was
LDWEIGHTS-bound — a 128x128 fp32r stationary load takes ~226ns, longer than
the matmul itself at NT=384 (~160ns), so the PE cadence was ~225ns/matmul.
bf16 halves the stationary load (~110ns), making the matmul compute the
binding constraint again, and also halves all DMA traffic (startup latency,
drain, chip-wide HBM contention). bf16 end-to-end rel err ~3e-3, far below
the 2e-2 gate (routing stays exact: gate+top4 are computed on host in fp32).

Per-core device layouts (everything pre-transposed on host for contiguous DMA):
  xT  [D, C] bf16 : routed tokens, transposed
  w1t [D, H] bf16 : w1[e].T
  w2t [H, O] bf16 : w2[e].T
  yT  [O, C] bf16 : expert output, transposed

Schedule notes:
  - Capacity is exact (max expert load, even-rounded), split into even tile
    widths in [256, 512] (PSUM bank caps a matmul's moving dim at 512 fp32).
  - Weights are DMAed in 128-wide column slices (separate tiles) so GEMM
    chains start as soon as their slice lands instead of after the full 8MB.
  - DMA emission order on the sync queue is hand-tuned: x0, w1 slices, x1,
    w2[0:8], x2, w2[8:16], x3, ... so the PE's program-order needs roughly
    track the FIFO queue's delivery order during the startup stream.
  - GEMM2(t) is emitted one tile behind GEMM1(t+1) (depth-1 software
    pipeline) to give the PE GEMM1 work while w2 is still streaming in.
  - y drains: PSUM -> SBUF copy on vector (cast to bf16), store DMA issued on
    scalar, so the sync queue (x + weights, latency-critical) is never
    blocked behind them.
"""

import numpy as np
import ml_dtypes
from contextlib import ExitStack

import concourse.bass as bass
import concourse.tile as tile
from concourse import bacc, mybir
from concourse import bass_utils

F32 = mybir.dt.float32
BF16 = mybir.dt.bfloat16
P = 128

TOP_K = 4
D, H, O, E = 2048, 1024, 2048, 8
_NC_CACHE = {}


def _tile_widths(C, target):
    """Split C tokens (padded to even) into even tiles <= 512 (the PSUM bank
    cap on a single matmul's moving dim). First and last tiles are small
    (256): the first starts compute after only ~1.5MB of DMA, the last
    shrinks the output-store drain after the final matmul."""
    C = max(C + (C % 2), 256)
    if C <= 768:
        C2 = C // 2
        ntiles = max(-(-C // target), 1)
        base = C2 // ntiles
        rem = C2 - base * ntiles
        widths = [2 * (base + 1)] * rem + [2 * base] * (ntiles - rem)
        widths.sort(reverse=True)
        return widths
    mid = C - 512
    ntiles = -(-mid // target)
    base = (mid // 2) // ntiles
    rem = mid // 2 - base * ntiles
    widths = [2 * (base + 1)] * rem + [2 * base] * (ntiles - rem)
    widths.sort(reverse=True)
    widths = [256] + widths + [256]
    assert sum(widths) == C and all(128 <= w <= 512 and w % 2 == 0 for w in widths)
    return widths


def build_expert_kernel(C, target):
    """Per-core program: dense [C, D] @ [D, H] -> relu -> @ [H, O] in bf16."""
    DC, HC, OC = D // P, H // P, O // P
    widths = _tile_widths(C, target)
    starts = [sum(widths[:i]) for i in range(len(widths))]
    NTILES = len(widths)
    NTMAX = max(widths)
    nc = bacc.Bacc("TRN2", target_bir_lowering=False, debug=False, num_devices=E)
    xT = nc.dram_tensor("xT", [D, C], BF16, kind="ExternalInput").ap()
    w1t = nc.dram_tensor("w1t", [D, H], BF16, kind="ExternalInput").ap()
    w2t = nc.dram_tensor("w2t", [H, O], BF16, kind="ExternalInput").ap()
    yT = nc.dram_tensor("yT", [O, C], BF16, kind="ExternalOutput").ap()

    with tile.TileContext(nc) as tc, ExitStack() as ctx:
        wpool = ctx.enter_context(tc.tile_pool(name="w", bufs=1))
        xpool = ctx.enter_context(tc.tile_pool(name="x", bufs=4))
        hpool = ctx.enter_context(tc.tile_pool(name="h", bufs=4))
        ypool = ctx.enter_context(tc.tile_pool(name="y", bufs=4))
        ps1 = ctx.enter_context(tc.tile_pool(name="ps1", bufs=4, space="PSUM"))
        ps2 = ctx.enter_context(tc.tile_pool(name="ps2", bufs=4, space="PSUM"))

        x_tiles = {}

        def dma_x(t):
            w_t = widths[t]
            x_t = xpool.tile([P, DC, NTMAX], BF16, name="x_t")[:, :, :w_t]
            nc.sync.dma_start(
                x_t[:],
                xT[:, starts[t]:starts[t] + w_t].rearrange("(dc p) n -> p dc n", p=P),
            )
            x_tiles[t] = x_t

        # --- PE warmup: dependency-free junk matmuls (uninitialized SBUF,
        # result never read) keep the Tensor engine continuously busy from
        # t~0 so the DVFS p-state ramps to full clock while the startup DMA
        # stream is still in flight ---
        wu_w = wpool.tile([P, P], BF16, name="wu_w")
        wu_x = wpool.tile([P, NTMAX], BF16, name="wu_x")
        wu_p = ps1.tile([P, NTMAX], F32, name="ph")
        nc.vector.memset(wu_w[:], 0.0)
        nc.vector.memset(wu_x[:], 0.0)
        for i in range(20):
            nc.tensor.matmul(wu_p[:], wu_w[:], wu_x[:], start=(i == 0),
                             stop=(i == 19))

        # --- startup DMA stream, hand-ordered for the FIFO queue: w1s[0]
        # plus the (small) first x tile unblock the first GEMM chain after
        # ~1.5MB; then the rest of w1, two more x tiles, and all of w2 ---
        w1s = []

        def dma_w1(hc):
            w = wpool.tile([P, DC, P], BF16, name=f"w1s{hc}")
            nc.sync.dma_start(
                w[:],
                w1t[:, hc * P:(hc + 1) * P].rearrange("(dc p) h -> p dc h", p=P),
            )
            w1s.append(w)

        dma_w1(0)
        dma_x(0)
        for hc in range(1, HC):
            dma_w1(hc)
        if NTILES > 1:
            dma_x(1)
        if NTILES > 2:
            dma_x(2)
        w2s = []

        def dma_w2(oc):
            w = wpool.tile([P, HC, P], BF16, name=f"w2s{oc}")
            nc.sync.dma_start(
                w[:],
                w2t[:, oc * P:(oc + 1) * P].rearrange("(hc p) o -> p hc o", p=P),
            )
            w2s.append(w)

        for oc in range(OC):
            dma_w2(oc)

        def gemm1(ts):
            """Fused GEMM1 over a group of token tiles: the dc loop is outer,
            the tile loop inner, so consecutive matmuls share the stationary
            w1 block (amortizes the PE weight-swap bubble)."""
            hs = {}
            for t in ts:
                w_t = widths[t]
                hs[t] = hpool.tile([P, HC, NTMAX], BF16, name="h_t")[:, :, :w_t]
            phs = {}
            for hc in range(HC):
                for t in ts:
                    phs[t] = ps1.tile([P, NTMAX], F32, name="ph")[:, :widths[t]]
                for dc in range(DC):
                    for t in ts:
                        nc.tensor.matmul(
                            phs[t][:], w1s[hc][:, dc, :], x_tiles[t][:, dc, :],
                            start=(dc == 0), stop=(dc == DC - 1),
                        )
                for t in ts:
                    nc.scalar.activation(
                        hs[t][:, hc, :], phs[t][:],
                        mybir.ActivationFunctionType.Relu,
                    )
            for t in ts:
                x_tiles.pop(t)
                h_tiles[t] = hs[t]

        def gemm2(ts, late=False):
            """Fused GEMM2 over a group of token tiles (same-stationary).
            Late groups issue their y stores on the sync queue, which is idle
            once the input stream has finished, so the drain after the final
            matmul is split across two DMA queues."""
            hs = {t: h_tiles.pop(t) for t in ts}
            pos = {}
            for oc in range(OC):
                for t in ts:
                    pos[t] = ps2.tile([P, NTMAX], F32, name="po")[:, :widths[t]]
                for hc in range(HC):
                    for t in ts:
                        nc.tensor.matmul(
                            pos[t][:], w2s[oc][:, hc, :], hs[t][:, hc, :],
                            start=(hc == 0), stop=(hc == HC - 1),
                        )
                for t in ts:
                    w_t = widths[t]
                    y_t = ypool.tile([P, NTMAX], BF16, name="y_t")[:, :w_t]
                    nc.vector.tensor_copy(y_t[:], pos[t][:])
                    eng = nc.sync if late and oc % 2 == 0 else nc.scalar
                    eng.dma_start(
                        yT[oc * P:(oc + 1) * P, starts[t]:starts[t] + w_t],
                        y_t[:],
                    )

        # --- group tiles: tile 0 alone (starts as soon as x0+w1s[0] land),
        # the rest in pairs; GEMM2 runs one group behind GEMM1 so the PE has
        # GEMM1 work while w2 streams in ---
        groups = [(0,)] + [
            tuple(range(t, min(t + 2, NTILES))) for t in range(1, NTILES, 2)
        ]
        h_tiles = {}
        ngroups = len(groups)
        for gi, g in enumerate(groups):
            if gi + 1 < ngroups and gi >= 1:
                for t in groups[gi + 1]:
                    dma_x(t)
            gemm1(g)
            if gi >= 1:
                gemm2(groups[gi - 1], late=(gi >= ngroups - 2))
        gemm2(groups[-1], late=True)
    nc.compile()
    return nc


def _route(xt, wg):
    """Host-side gate + top-4. Gap between 4th/5th gate values is ~3e-5 for
    this distribution, far above fp32 matmul noise, so fp32 reproduces the
    reference top-k set exactly."""
    gate = xt @ wg  # [N, E] fp32
    top4 = np.argpartition(-gate, TOP_K - 1, axis=1)[:, :TOP_K]  # set, unordered
    return top4


def kernel(x, wg, w1, w2, _want_results=False, _run_kwargs=None):
    x = np.asarray(x, dtype=np.float32)
    wg = np.asarray(wg, dtype=np.float32)
    w1 = np.asarray(w1, dtype=np.float32)
    w2 = np.asarray(w2, dtype=np.float32)
    B, S, Dx = x.shape
    N = B * S
    xt = np.ascontiguousarray(x.reshape(N, Dx))
    top4 = _route(xt, wg)

    # token lists per expert
    sel = np.zeros((N, E), dtype=bool)
    np.put_along_axis(sel, top4, True, axis=1)
    tokens = [np.nonzero(sel[:, e])[0] for e in range(E)]
    counts = np.array([len(t) for t in tokens])
    CAP = max(int(counts.max()), 256)
    CAP += CAP % 2

    if CAP not in _NC_CACHE:
        last_err = None
        for target in (512, 448, 384):
            try:
                _NC_CACHE[CAP] = build_expert_kernel(CAP, target)
                break
            except ValueError as err:  # SBUF pool allocation failure
                last_err = err
        else:
            raise last_err
    nc = _NC_CACHE[CAP]

    xtb = xt.astype(ml_dtypes.bfloat16)
    in_maps = []
    for e in range(E):
        xe = np.zeros((CAP, Dx), dtype=ml_dtypes.bfloat16)
        xe[:counts[e]] = xtb[tokens[e]]
        in_maps.append({
            "xT": np.ascontiguousarray(xe.T),
            "w1t": np.ascontiguousarray(w1[e].T.astype(ml_dtypes.bfloat16)),
            "w2t": np.ascontiguousarray(w2[e].T.astype(ml_dtypes.bfloat16)),
        })

    res = bass_utils.run_bass_kernel_spmd(
        nc, in_maps, core_ids=list(range(E)), **(_run_kwargs or {})
    )

    out = np.zeros((N, O), dtype=np.float32)
    for e in range(E):
        out[tokens[e]] += res.results[e]["yT"].T[:counts[e]].astype(np.float32)
    out = out.reshape(B, S, O)
    if _want_results:
        return out, res
    return out


# revision 12
# speedup vs baseline: 1.0231x; 1.0231x over previous
"""MoE (dense-act-dense, top-4 of 8 experts) Trainium2 kernel.

Strategy (expert-parallel, host-side dispatch):
  - The forward combine weight is exactly 1.0 (straight-through gate trick in
    the reference), so out[n] = sum_{e in top4(n)} expert_e(x[n]).
  - Host computes the tiny gate matmul + top-4 routing (0.05% of FLOPs) and
    dispatches tokens: core e receives the tokens routed to expert e
    (capacity-padded), plus expert e's weights. This is the sharding step.
  - Each of the 8 cores runs a dense 2-layer MLP (relu between) on its tokens:
      h = relu(w1[e] @ x) ; y = w2[e] @ h
    as two chained GEMMs in bf16 (fp32 PSUM accumulate).
  - Host scatter-adds per-expert outputs back (weight 1.0 per selection).

Why bf16 (not fp32r): trace analysis showed the fp32r kernel was
LDWEIGHTS-bound — a 128x128 fp32r stationary load takes ~226ns, longer than
the matmul itself at NT=384 (~160ns), so the PE cadence was ~225ns/matmul.
bf16 halves the stationary load (~110ns), making the matmul compute the
binding constraint again, and also halves all DMA traffic (startup latency,
drain, chip-wide HBM contention). bf16 end-to-end rel err ~3e-3, far below
the 2e-2 gate (routing stays exact: gate+top4 are computed on host in fp32).

Per-core device layouts (everything pre-transposed on host for contiguous DMA):
  xT  [D, C] bf16 : routed tokens, transposed
  w1t [D, H] bf16 : w1[e].T
  w2t [H, O] bf16 : w2[e].T
  yT  [O, C] bf16 : expert output, transposed

Schedule notes:
  - Capacity is exact (max expert load, even-rounded), split into even tile
    widths in [256, 512] (PSUM bank caps a matmul's moving dim at 512 fp32).
  - Weights are DMAed in 128-wide column slices (separate tiles) so GEMM
    chains start as soon as their slice lands instead of after the full 8MB.
  - DMA emission order on the sync queue is hand-tuned: x0, w1 slices, x1,
    w2[0:8], x2, w2[8:16], x3, ... so the PE's program-order needs roughly
    track the FIFO queue's delivery order during the startup stream.
  - GEMM2(t) is emitted one tile behind GEMM1(t+1) (depth-1 software
    pipeline) to give the PE GEMM1 work while w2 is still streaming in.
  - y drains: PSUM -> SBUF copy on vector (cast to bf16), store DMA issued on
    scalar, so the sync queue (x + weights, latency-critical) is never
    blocked behind them.
"""

import numpy as np
import ml_dtypes
from contextlib import ExitStack

import concourse.bass as bass
import concourse.tile as tile
from concourse import bacc, mybir
from concourse import bass_utils

F32 = mybir.dt.float32
BF16 = mybir.dt.bfloat16
P = 128

TOP_K = 4
D, H, O, E = 2048, 1024, 2048, 8
_NC_CACHE = {}


def _tile_widths(C, target):
    """Split C tokens (padded to even) into even tiles of near-equal width in
    [256, 512]. 512 is the PSUM bank cap on a single matmul's moving dim.
    Uniform large tiles beat a small lead-in tile: the first tile's GEMM1 is
    the PE's only cover while the rest of w1/x/w2 stream in, so shrinking it
    just moves the idle later."""
    C = max(C + (C % 2), 256)
    C2 = C // 2
    ntiles = min(-(-C // target), C2 // 128)
    base = C2 // ntiles
    rem = C2 - base * ntiles
    widths = [2 * (base + 1)] * rem + [2 * base] * (ntiles - rem)
    widths.sort(reverse=True)
    assert sum(widths) == C and all(256 <= w <= 512 and w % 2 == 0 for w in widths)
    return widths


def build_expert_kernel(C, target):
    """Per-core program: dense [C, D] @ [D, H] -> relu -> @ [H, O] in bf16."""
    DC, HC, OC = D // P, H // P, O // P
    widths = _tile_widths(C, target)
    starts = [sum(widths[:i]) for i in range(len(widths))]
    NTILES = len(widths)
    NTMAX = max(widths)
    nc = bacc.Bacc("TRN2", target_bir_lowering=False, debug=False, num_devices=E)
    xT = nc.dram_tensor("xT", [D, C], BF16, kind="ExternalInput").ap()
    w1t = nc.dram_tensor("w1t", [D, H], BF16, kind="ExternalInput").ap()
    w2t = nc.dram_tensor("w2t", [H, O], BF16, kind="ExternalInput").ap()
    yT = nc.dram_tensor("yT", [O, C], BF16, kind="ExternalOutput").ap()

    with tile.TileContext(nc) as tc, ExitStack() as ctx:
        wpool = ctx.enter_context(tc.tile_pool(name="w", bufs=1))
        xpool = ctx.enter_context(tc.tile_pool(name="x", bufs=4))
        hpool = ctx.enter_context(tc.tile_pool(name="h", bufs=4))
        ypool = ctx.enter_context(tc.tile_pool(name="y", bufs=4))
        ps1 = ctx.enter_context(tc.tile_pool(name="ps1", bufs=4, space="PSUM"))
        ps2 = ctx.enter_context(tc.tile_pool(name="ps2", bufs=4, space="PSUM"))

        x_tiles = {}

        def dma_x(t):
            w_t = widths[t]
            x_t = xpool.tile([P, DC, NTMAX], BF16, name="x_t")[:, :, :w_t]
            nc.sync.dma_start(
                x_t[:],
                xT[:, starts[t]:starts[t] + w_t].rearrange("(dc p) n -> p dc n", p=P),
            )
            x_tiles[t] = x_t

        # --- PE warmup: dependency-free junk matmuls (uninitialized SBUF,
        # result never read) keep the Tensor engine continuously busy from
        # t~0 so the DVFS p-state ramps to full clock while the startup DMA
        # stream is still in flight ---
        wu_w = wpool.tile([P, P], BF16, name="wu_w")
        wu_x = wpool.tile([P, NTMAX], BF16, name="wu_x")
        wu_p = ps1.tile([P, NTMAX], F32, name="ph")
        nc.vector.memset(wu_w[:], 0.0)
        nc.vector.memset(wu_x[:], 0.0)
        for i in range(20):
            nc.tensor.matmul(wu_p[:], wu_w[:], wu_x[:], start=(i == 0),
                             stop=(i == 19))

        # --- startup DMA stream, hand-ordered for the FIFO queue: w1s[0]
        # plus the (small) first x tile unblock the first GEMM chain after
        # ~1.5MB; then the rest of w1, two more x tiles, and all of w2 ---
        w1s = []

        def dma_w1(hc):
            w = wpool.tile([P, DC, P], BF16, name=f"w1s{hc}")
            nc.sync.dma_start(
                w[:],
                w1t[:, hc * P:(hc + 1) * P].rearrange("(dc p) h -> p dc h", p=P),
            )
            w1s.append(w)

        dma_w1(0)
        dma_x(0)
        for hc in range(1, HC):
            dma_w1(hc)
        if NTILES > 1:
            dma_x(1)
        if NTILES > 2:
            dma_x(2)
        w2s = []

        def dma_w2(oc):
            w = wpool.tile([P, HC, P], BF16, name=f"w2s{oc}")
            nc.sync.dma_start(
                w[:],
                w2t[:, oc * P:(oc + 1) * P].rearrange("(hc p) o -> p hc o", p=P),
            )
            w2s.append(w)

        for oc in range(OC):
            dma_w2(oc)

        def gemm1(ts):
            """Fused GEMM1 over a group of token tiles: the dc loop is outer,
            the tile loop inner, so consecutive matmuls share the stationary
            w1 block (amortizes the PE weight-swap bubble)."""
            hs = {}
            for t in ts:
                w_t = widths[t]
                hs[t] = hpool.tile([P, HC, NTMAX], BF16, name="h_t")[:, :, :w_t]
            phs = {}
            for hc in range(HC):
                for t in ts:
                    phs[t] = ps1.tile([P, NTMAX], F32, name="ph")[:, :widths[t]]
                for dc in range(DC):
                    for t in ts:
                        nc.tensor.matmul(
                            phs[t][:], w1s[hc][:, dc, :], x_tiles[t][:, dc, :],
                            start=(dc == 0), stop=(dc == DC - 1),
                        )
                for t in ts:
                    nc.scalar.activation(
                        hs[t][:, hc, :], phs[t][:],
                        mybir.ActivationFunctionType.Relu,
                    )
            for t in ts:
                x_tiles.pop(t)
                h_tiles[t] = hs[t]

        def gemm2(ts, late=False):
            """Fused GEMM2 over a group of token tiles (same-stationary).
            Late groups issue their y stores on the sync queue, which is idle
            once the input stream has finished, so the drain after the final
            matmul is split across two DMA queues."""
            hs = {t: h_tiles.pop(t) for t in ts}
            pos = {}
            for oc in range(OC):
                for t in ts:
                    pos[t] = ps2.tile([P, NTMAX], F32, name="po")[:, :widths[t]]
                for hc in range(HC):
                    for t in ts:
                        nc.tensor.matmul(
                            pos[t][:], w2s[oc][:, hc, :], hs[t][:, hc, :],
                            start=(hc == 0), stop=(hc == HC - 1),
                        )
                for t in ts:
                    w_t = widths[t]
                    y_t = ypool.tile([P, NTMAX], BF16, name="y_t")[:, :w_t]
                    nc.vector.tensor_copy(y_t[:], pos[t][:])
                    eng = nc.sync if late and oc % 2 == 0 else nc.scalar
                    eng.dma_start(
                        yT[oc * P:(oc + 1) * P, starts[t]:starts[t] + w_t],
                        y_t[:],
                    )

        # --- group tiles: tile 0 alone (starts as soon as x0+w1s[0] land),
        # the rest in pairs; GEMM2 runs one group behind GEMM1 so the PE has
        # GEMM1 work while w2 streams in ---
        groups = [(0,)] + [
            tuple(range(t, min(t + 2, NTILES))) for t in range(1, NTILES, 2)
        ]
        h_tiles = {}
        ngroups = len(groups)
        for gi, g in enumerate(groups):
            if gi + 1 < ngroups and gi >= 1:
                for t in groups[gi + 1]:
                    dma_x(t)
            gemm1(g)
            if gi >= 1:
                gemm2(groups[gi - 1], late=(gi >= ngroups - 2))
        gemm2(groups[-1], late=True)
    nc.compile()
    return nc


def _route(xt, wg):
    """Host-side gate + top-4. Gap between 4th/5th gate values is ~3e-5 for
    this distribution, far above fp32 matmul noise, so fp32 reproduces the
    reference top-k set exactly."""
    gate = xt @ wg  # [N, E] fp32
    top4 = np.argpartition(-gate, TOP_K - 1, axis=1)[:, :TOP_K]  # set, unordered
    return top4


def kernel(x, wg, w1, w2, _want_results=False, _run_kwargs=None):
    x = np.asarray(x, dtype=np.float32)
    wg = np.asarray(wg, dtype=np.float32)
    w1 = np.asarray(w1, dtype=np.float32)
    w2 = np.asarray(w2, dtype=np.float32)
    B, S, Dx = x.shape
    N = B * S
    xt = np.ascontiguousarray(x.reshape(N, Dx))
    top4 = _route(xt, wg)

    # token lists per expert
    sel = np.zeros((N, E), dtype=bool)
    np.put_along_axis(sel, top4, True, axis=1)
    tokens = [np.nonzero(sel[:, e])[0] for e in range(E)]
    counts = np.array([len(t) for t in tokens])
    CAP = max(int(counts.max()), 256)
    CAP += CAP % 2

    if CAP not in _NC_CACHE:
        last_err = None
        for target in (512, 448, 384):
            try:
                _NC_CACHE[CAP] = build_expert_kernel(CAP, target)
                break
            except ValueError as err:  # SBUF pool allocation failure
                last_err = err
        else:
            raise last_err
    nc = _NC_CACHE[CAP]

    xtb = xt.astype(ml_dtypes.bfloat16)
    in_maps = []
    for e in range(E):
        xe = np.zeros((CAP, Dx), dtype=ml_dtypes.bfloat16)
        xe[:counts[e]] = xtb[tokens[e]]
        in_maps.append({
            "xT": np.ascontiguousarray(xe.T),
            "w1t": np.ascontiguousarray(w1[e].T.astype(ml_dtypes.bfloat16)),
            "w2t": np.ascontiguousarray(w2[e].T.astype(ml_dtypes.bfloat16)),
        })

    res = bass_utils.run_bass_kernel_spmd(
        nc, in_maps, core_ids=list(range(E)), **(_run_kwargs or {})
    )

    out = np.zeros((N, O), dtype=np.float32)
    for e in range(E):
        out[tokens[e]] += res.results[e]["yT"].T[:counts[e]].astype(np.float32)
    out = out.reshape(B, S, O)
    if _want_results:
        return out, res
    return out
